# revision 9
# baseline (speedup 1.0000x reference)
"""Trainium2 Bass kernel for nn_MultiHeadAttention_87110526697836.

Strategy: data-parallel over batch B=8 across the 8 NeuronCores (one batch
element per core, no collectives). Per core, a causal MHA layer with
relative-position biases (max_dist=16), fused softmax, output projection,
residual and LayerNorm.

Layout: activations kept transposed ([U, T], "T-plan") so that
  - QKV projections use natural weight layouts,
  - scores are computed as [s-partition, t-free] tiles (kh stationary),
  - P@V consumes the score tiles directly (no P transposes),
  - per-t softmax scalars (1/den) are applied after P@V where t is back on
    partitions.
The relative-position band (17 diagonals) is handled via small DRAM
"skew staging" round trips: diagonal bands of a [128, T] tile are
extracted/scattered with strided DMA access patterns on flat DRAM buffers.
"""

import math

import numpy as np
import ml_dtypes

import bass_rust
import concourse.bass as bass
import concourse.mybir as mybir
import concourse.tile as tile
from concourse.bass_utils import run_bass_kernel_spmd

F32 = mybir.dt.float32
BF16 = mybir.dt.bfloat16

H = 8
M = 16
U = 512
DH = U // H        # 64
T = 1024
B = 8
EPS = 1e-3
NT = T // 128      # 8 t-chunks / s-chunks
WTW = 1040         # padded wT tile width (T + 16)
EXT_ROWS, EXT_W = 160, 144   # staging slot geometry
SLOT = EXT_ROWS * EXT_W

# ---------------------------------------------------------------------------
# Tile-framework workarounds: walrus accepts a limited number of sync-wait
# commands per instruction; split excess waits onto same-engine NOPs.
# ---------------------------------------------------------------------------


def _patched_drain_and_barrier(self, tick_clock, wait_clock):
    from bass_rust import ScopedClock

    nc = self.nc
    nop_inst = nc.sync.nop()
    wait_clock.add_sem_waits(nop_inst.ins, ScopedClock({None: tick_clock.global_clock}))
    si = nop_inst.ins.sync_info
    if si is not None and si.on_wait and len(si.on_wait) > 1:
        waits = list(si.on_wait)
        nop_inst.ins.sync_info = bass_rust.SyncInfo(
            on_wait=[waits[0]], on_update=list(si.on_update or [])
        )
        for w in waits[1:]:
            extra = nc.sync.nop()
            extra.ins.sync_info = bass_rust.SyncInfo(on_wait=[w], on_update=[])
    nc.sync.drain()
    nc.all_engine_barrier()
    popped = nc._tile_sem_poison_stack.pop()
    assert popped is self._sem_poison
    nc.clear_and_free_semaphores(list(self.sems.allocated().values()))
    nc.all_engine_barrier()


tile.TileContext._drain_and_barrier = _patched_drain_and_barrier


def split_excess_waits(nc, limit=1):
    counter = 0
    for f in nc.m.functions:
        for bb in f.blocks:
            insts = bb.instructions
            out = []
            changed = False
            for ins in insts:
                si = ins.sync_info
                if si is not None and si.on_wait is not None and len(si.on_wait) > limit:
                    waits = list(si.on_wait)
                    extra, keep = waits[:-limit], waits[-limit:]
                    for w in extra:
                        counter += 1
                        nop = mybir.InstNoOp(name=f"waitsplit-{counter}", ins=[], outs=[])
                        nop.engine = ins.engine
                        nop.sync_info = bass_rust.SyncInfo(on_wait=[w], on_update=[])
                        nc.inst_map[nop.name] = nop
                        out.append(nop)
                    ins.sync_info = bass_rust.SyncInfo(
                        on_wait=keep, on_update=list(si.on_update or [])
                    )
                    changed = True
                out.append(ins)
            if changed:
                bb.instructions = out
    return counter


def dram_ap(t, offset, dims):
    return bass.AP(tensor=t, offset=offset, ap=[list(d) for d in dims])


# ---------------------------------------------------------------------------
# Kernel builder
# ---------------------------------------------------------------------------


def build_nc(apply_gamma_beta: bool):
    nc = bass.Bass()

    # ---- external tensors -------------------------------------------------
    p_qT = nc.declare_dram_parameter("qT", [U, T], BF16, isOutput=False)
    p_kT = nc.declare_dram_parameter("kT", [U, T], BF16, isOutput=False)
    p_vT = nc.declare_dram_parameter("vT", [U, T], BF16, isOutput=False)
    p_qn = nc.declare_dram_parameter("qn", [T, U], BF16, isOutput=False)
    p_Wq = nc.declare_dram_parameter("Wq8", [U, U], BF16, isOutput=False)
    p_Wk = nc.declare_dram_parameter("Wk", [U, U], BF16, isOutput=False)
    p_Wv = nc.declare_dram_parameter("Wv", [U, U], BF16, isOutput=False)
    p_Wo = nc.declare_dram_parameter("Wo", [U, U], BF16, isOutput=False)
    p_bq = nc.declare_dram_parameter("bq8", [U, 1], F32, isOutput=False)
    p_bk = nc.declare_dram_parameter("bk", [U, 1], F32, isOutput=False)
    p_bv = nc.declare_dram_parameter("bv_row", [1, U], BF16, isOutput=False)
    p_bo = nc.declare_dram_parameter("bo_row", [1, U], BF16, isOutput=False)
    p_pk0 = nc.declare_dram_parameter("pe_k0t", [U, 1], F32, isOutput=False)
    p_dpk = nc.declare_dram_parameter("dpe_kT", [DH, M], BF16, isOutput=False)
    p_dpv = nc.declare_dram_parameter("dpe_v", [M, DH], BF16, isOutput=False)
    p_gam = nc.declare_dram_parameter("gamma_r", [1, U], F32, isOutput=False)
    p_bet = nc.declare_dram_parameter("beta_r", [1, U], F32, isOutput=False)
    p_out = nc.declare_dram_parameter("out", [T, U], F32, isOutput=True)

    ext_stage = nc.dram_tensor("ext_stage", [H * NT * SLOT], BF16)
    rden_stage = nc.dram_tensor("rden_stage", [H * 2 * 512], F32)
    corr_stage = nc.dram_tensor("corr_stage", [H * NT * SLOT], BF16)

    with tile.TileContext(nc) as tc:
        import contextlib

        cstack = contextlib.ExitStack()
        consts = cstack.enter_context(tc.tile_pool(name="consts", bufs=1))
        inp_pool = cstack.enter_context(tc.tile_pool(name="inp", bufs=1))
        emb_pool = cstack.enter_context(tc.tile_pool(name="emb", bufs=1))
        wt_pool = cstack.enter_context(tc.tile_pool(name="wt", bufs=12))
        band_pool = cstack.enter_context(tc.tile_pool(name="band", bufs=24))
        g_pool = cstack.enter_context(tc.tile_pool(name="g", bufs=3))
        dense_pool = cstack.enter_context(tc.tile_pool(name="dense", bufs=8))
        mrg_pool = cstack.enter_context(tc.tile_pool(name="mrg", bufs=1))
        nrm_pool = cstack.enter_context(tc.tile_pool(name="nrm", bufs=6))
        ln_pool = cstack.enter_context(tc.tile_pool(name="ln", bufs=4))
        zpool = cstack.enter_context(tc.tile_pool(name="z", bufs=1))

        ps_a = cstack.enter_context(tc.tile_pool(name="ps_a", bufs=2, space="PSUM"))
        ps_sc = cstack.enter_context(tc.tile_pool(name="ps_sc", bufs=3, space="PSUM"))
        ps_o = cstack.enter_context(tc.tile_pool(name="ps_o", bufs=2, space="PSUM"))

        # ---- staging zero-init -------------------------------------------
        ZW = 2304
        ztile = zpool.tile([128, ZW], BF16)
        nc.vector.memset(ztile, 0.0)
        total = H * NT * SLOT
        chunk = 128 * ZW
        assert total % chunk == 0
        for off in range(0, total, chunk):
            nc.gpsimd.dma_start(
                out=dram_ap(ext_stage, off, [[ZW, 128], [1, ZW]]),
                in_=ztile,
            )
            nc.gpsimd.dma_start(
                out=dram_ap(corr_stage, off, [[ZW, 128], [1, ZW]]),
                in_=ztile,
            )

        # ---- constant loads ----------------------------------------------
        def load_w(param):
            tiles = []
            for ci in range(4):
                t_ = consts.tile([128, U], BF16, tag=f"w{param.name}{ci}")
                nc.sync.dma_start(out=t_, in_=param[ci * 128:(ci + 1) * 128, :])
                tiles.append(t_)
            return tiles

        Wq_sb = load_w(p_Wq)
        Wk_sb = load_w(p_Wk)
        Wv_sb = load_w(p_Wv)
        Wo_sb = load_w(p_Wo)

        def load_xT(param, name):
            tiles = []
            for ci in range(4):
                t_ = inp_pool.tile([128, T], BF16, tag=f"x{name}{ci}")
                nc.sync.dma_start(out=t_, in_=param[ci * 128:(ci + 1) * 128, :])
                tiles.append(t_)
            return tiles

        qT_sb = load_xT(p_qT, "q")
        kT_sb = load_xT(p_kT, "k")
        vT_sb = load_xT(p_vT, "v")

        bq_sb, bk_sb, pk0_sb = [], [], []
        for ci in range(4):
            t_ = consts.tile([128, 1], F32, tag=f"bq{ci}")
            nc.sync.dma_start(out=t_, in_=p_bq[ci * 128:(ci + 1) * 128, :])
            bq_sb.append(t_)
            t_ = consts.tile([128, 1], F32, tag=f"bk{ci}")
            nc.sync.dma_start(out=t_, in_=p_bk[ci * 128:(ci + 1) * 128, :])
            bk_sb.append(t_)
            t_ = consts.tile([128, 1], F32, tag=f"pk0{ci}")
            nc.sync.dma_start(out=t_, in_=p_pk0[ci * 128:(ci + 1) * 128, :])
            pk0_sb.append(t_)

        bv_sb = consts.tile([1, U], BF16, tag="bv")
        nc.sync.dma_start(out=bv_sb, in_=p_bv[:, :])
        bo_sb = consts.tile([1, U], BF16, tag="bo")
        nc.sync.dma_start(out=bo_sb, in_=p_bo[:, :])
        dpk_sb = consts.tile([128, M], BF16, tag="dpk")
        nc.sync.dma_start(out=dpk_sb[0:DH, :], in_=p_dpk[:, :])
        nc.sync.dma_start(out=dpk_sb[DH:128, :], in_=p_dpk[:, :])
        dpv_sb = consts.tile([M, DH], BF16, tag="dpv")
        nc.sync.dma_start(out=dpv_sb, in_=p_dpv[:, :])
        gam_sb = consts.tile([1, U], F32, tag="gam")
        nc.sync.dma_start(out=gam_sb, in_=p_gam[:, :])
        bet_sb = consts.tile([1, U], F32, tag="bet")
        nc.sync.dma_start(out=bet_sb, in_=p_bet[:, :])

        if apply_gamma_beta:
            gam_bc = consts.tile([128, U], F32, tag="gambc")
            nc.sync.dma_start(
                out=gam_bc, in_=dram_ap(p_gam, 0, [[0, 128], [1, U]])
            )
            bet_bc = consts.tile([128, U], F32, tag="betbc")
            nc.sync.dma_start(
                out=bet_bc, in_=dram_ap(p_bet, 0, [[0, 128], [1, U]])
            )

        ones_row = consts.tile([1, 128], BF16, tag="ones")
        nc.vector.memset(ones_row, 1.0)
        eps_sb = consts.tile([128, 1], F32, tag="eps")
        nc.vector.memset(eps_sb, EPS)

        # ---- phase 1: projections ----------------------------------------
        # q_embT / k_embT: [U, T] bf16 as 4 tiles of [128, T]
        q_embT, k_embT = [], []
        for co in range(4):
            t_ = emb_pool.tile([128, T], BF16, tag=f"qe{co}")
            q_embT.append(t_)
            t_ = emb_pool.tile([128, T], BF16, tag=f"ke{co}")
            k_embT.append(t_)
        v_pad = []
        for ti in range(NT):
            t_ = emb_pool.tile([128, H, 66], BF16, tag=f"vp{ti}")
            v_pad.append(t_)

        for co in range(4):
            for th in range(2):
                ps = ps_a.tile([128, 512], F32, tag="proj")
                for ci in range(4):
                    nc.tensor.matmul(
                        ps,
                        lhsT=Wq_sb[ci][:, co * 128:(co + 1) * 128],
                        rhs=qT_sb[ci][:, th * 512:(th + 1) * 512],
                        start=(ci == 0),
                        stop=(ci == 3),
                    )
                nc.scalar.activation(
                    out=q_embT[co][:, th * 512:(th + 1) * 512],
                    in_=ps,
                    func=mybir.ActivationFunctionType.Relu,
                    bias=bq_sb[co],
                )
        for co in range(4):
            for th in range(2):
                ps = ps_a.tile([128, 512], F32, tag="proj")
                for ci in range(4):
                    nc.tensor.matmul(
                        ps,
                        lhsT=Wk_sb[ci][:, co * 128:(co + 1) * 128],
                        rhs=kT_sb[ci][:, th * 512:(th + 1) * 512],
                        start=(ci == 0),
                        stop=(ci == 3),
                    )
                nc.scalar.activation(
                    out=k_embT[co][:, th * 512:(th + 1) * 512],
                    in_=ps,
                    func=mybir.ActivationFunctionType.Relu,
                    bias=bk_sb[co],
                )
            # fold pe_k[0] into keys (handles the r0 part of the rel-k bias)
            nc.vector.tensor_scalar_add(
                out=k_embT[co], in0=k_embT[co], scalar1=pk0_sb[co]
            )

        for ti in range(NT):
            ps = ps_a.tile([128, 512], F32, tag="proj")
            for ci in range(4):
                nc.tensor.matmul(
                    ps,
                    lhsT=vT_sb[ci][:, ti * 128:(ti + 1) * 128],
                    rhs=Wv_sb[ci],
                    start=(ci == 0),
                    stop=False,
                )
            nc.tensor.matmul(ps, lhsT=ones_row, rhs=bv_sb, start=False, stop=True)
            nc.scalar.activation(
                out=v_pad[ti][:, :, 0:64],
                in_=ps.rearrange("p (a b) -> p a b", a=H),
                func=mybir.ActivationFunctionType.Relu,
            )
            nc.vector.memset(v_pad[ti][:, :, 64:65], 1.0)

        # mergedT: [U, T] bf16 (normalized attention output, head-major rows)
        mergedT = []
        for co in range(4):
            t_ = mrg_pool.tile([128, T], BF16, tag=f"mg{co}")
            mergedT.append(t_)

        # ---- phase 2: attention per head ---------------------------------
        for h in range(H):
            qhT = q_embT[h // 2][64 * (h % 2):64 * (h % 2) + 64, :]
            khT = k_embT[h // 2][64 * (h % 2):64 * (h % 2) + 64, :]

            # G = exp(qh . (pe_k[c] - pe_k[0])) for c=1..16  -> [16, T] bf16
            g_sb = g_pool.tile([M, T], BF16, tag="gsb")
            for th in range(2):
                ps = ps_sc.tile([128, 512], F32, tag="scores")
                nc.tensor.matmul(
                    ps[0:M, :],
                    lhsT=dpk_sb[64 * (h % 2):64 * (h % 2) + 64, :],
                    rhs=qhT[:, th * 512:(th + 1) * 512],
                    start=True,
                    stop=True,
                )
                nc.scalar.activation(
                    out=g_sb[:, th * 512:(th + 1) * 512],
                    in_=ps[0:M, :],
                    func=mybir.ActivationFunctionType.Exp,
                )

            # scores + exp per s-chunk
            wT = []
            for c in range(NT):
                s0 = c * 128
                wt_t = wt_pool.tile([128, WTW], BF16, tag="wt")
                wT.append(wt_t)
                for th in range(2):
                    lo = max(th * 512, s0)
                    hi = (th + 1) * 512
                    if lo >= hi:
                        continue
                    width = hi - lo
                    ps = ps_sc.tile([128, 512], F32, tag="scores")
                    nc.tensor.matmul(
                        ps[:, 0:width],
                        lhsT=khT[:, s0:s0 + 128],
                        rhs=qhT[:, lo:hi],
                        start=True,
                        stop=True,
                    )
                    nc.scalar.activation(
                        out=wt_t[:, lo:lo + width],
                        in_=ps[:, 0:width],
                        func=mybir.ActivationFunctionType.Exp,
                    )
                # causal mask inside the diagonal block: zero where t' < s'
                nc.gpsimd.affine_select(
                    out=wt_t[:, s0:s0 + 128],
                    in_=wt_t[:, s0:s0 + 128],
                    compare_op=mybir.AluOpType.is_ge,
                    fill=0.0,
                    base=0,
                    channel_multiplier=-1,
                    pattern=[[1, 128]],
                )
                # stage the band window for diagonal extraction
                slot = (h * NT + c) * SLOT
                nc.gpsimd.dma_start(
                    out=dram_ap(ext_stage, slot + 15 * EXT_W, [[EXT_W, 128], [1, EXT_W]]),
                    in_=wt_t[:, s0:s0 + EXT_W],
                )

            # band extraction + multiplicative correction scatter
            wband = []
            for ti in range(NT):
                slot = (h * NT + ti) * SLOT
                bE = band_pool.tile([M, 128], BF16, tag="bE")
                nc.gpsimd.dma_start(
                    out=bE, in_=dram_ap(ext_stage, slot, [[EXT_W, M], [EXT_W + 1, 128]])
                )
                if ti >= 1:
                    slot_p = (h * NT + ti - 1) * SLOT
                    bE2 = band_pool.tile([M, 16], BF16, tag="bE2")
                    nc.gpsimd.dma_start(
                        out=bE2,
                        in_=dram_ap(
                            ext_stage,
                            slot_p + EXT_W * 128 + 128,
                            [[EXT_W, M], [EXT_W + 1, 16]],
                        ),
                    )
                    nc.vector.tensor_tensor(
                        out=bE[:, 0:16], in0=bE[:, 0:16], in1=bE2,
                        op=mybir.AluOpType.add,
                    )
                wb = band_pool.tile([M, 128], BF16, tag="wb")
                nc.vector.tensor_tensor(
                    out=wb, in0=bE, in1=g_sb[:, ti * 128:(ti + 1) * 128],
                    op=mybir.AluOpType.mult,
                )
                wband.append(wb)
                co_t = band_pool.tile([M, 128], BF16, tag="corr")
                nc.vector.tensor_tensor(
                    out=co_t, in0=wb, in1=bE, op=mybir.AluOpType.subtract
                )
                # scatter correction back: slot(ti) main part
                nc.gpsimd.dma_start(
                    out=dram_ap(
                        corr_stage, slot, [[EXT_W, M], [EXT_W + 1, 128]]
                    ),
                    in_=co_t,
                )
                if ti >= 1:
                    slot_p = (h * NT + ti - 1) * SLOT
                    nc.gpsimd.dma_start(
                        out=dram_ap(
                            corr_stage,
                            slot_p + EXT_W * 128 + 128,
                            [[EXT_W, M], [EXT_W + 1, 16]],
                        ),
                        in_=co_t[:, 0:16],
                    )

            # dense correction add onto each w tile
            for c in range(NT):
                s0 = c * 128
                slot = (h * NT + c) * SLOT
                cd = dense_pool.tile([128, EXT_W], BF16, tag="cd")
                nc.gpsimd.dma_start(
                    out=cd,
                    in_=dram_ap(corr_stage, slot + 15 * EXT_W, [[EXT_W, 128], [1, EXT_W]]),
                )
                nc.vector.tensor_tensor(
                    out=wT[c][:, s0:s0 + EXT_W],
                    in0=wT[c][:, s0:s0 + EXT_W],
                    in1=cd,
                    op=mybir.AluOpType.add,
                )

            # P @ [V | 1] and rel-v band term; normalize into mergedT
            for th in range(2):
                po = ps_o.tile([65, 512], F32, tag="po")
                nmax = 4 * (th + 1)
                for c in range(nmax):
                    s0 = c * 128
                    lo = max(0, s0 - th * 512)
                    nc.tensor.matmul(
                        po[:, lo:512],
                        lhsT=v_pad[c][:, h, 0:65],
                        rhs=wT[c][:, th * 512 + lo:(th + 1) * 512],
                        start=(c == 0),
                        stop=False,
                        skip_group_check=True,
                    )
                for tl in range(4):
                    ti = th * 4 + tl
                    nc.tensor.matmul(
                        po[0:64, tl * 128:(tl + 1) * 128],
                        lhsT=dpv_sb,
                        rhs=wband[ti],
                        start=False,
                        stop=(tl == 3),
                        skip_group_check=True,
                    )
                mslice = mergedT[h // 2][64 * (h % 2):64 * (h % 2) + 64,
                                        th * 512:(th + 1) * 512]
                nrm = nrm_pool.tile([1, 512], F32, tag="rden")
                nc.vector.reciprocal(out=nrm, in_=po[64:65, :])
                roff = (h * 2 + th) * 512
                nc.gpsimd.dma_start(
                    out=dram_ap(rden_stage, roff, [[512, 1], [1, 512]]), in_=nrm
                )
                nrmb = nrm_pool.tile([64, 512], F32, tag="rdenb")
                nc.gpsimd.dma_start(
                    out=nrmb, in_=dram_ap(rden_stage, roff, [[0, 64], [1, 512]])
                )
                nc.vector.tensor_tensor(
                    out=mslice,
                    in0=po[0:64, :],
                    in1=nrmb,
                    op=mybir.AluOpType.mult,
                )

        # ---- phase 3: output projection + residual + layernorm ----------
        for ti in range(NT):
            ps = ps_a.tile([128, 512], F32, tag="proj")
            for ci in range(4):
                nc.tensor.matmul(
                    ps,
                    lhsT=mergedT[ci][:, ti * 128:(ti + 1) * 128],
                    rhs=Wo_sb[ci],
                    start=(ci == 0),
                    stop=False,
                )
            nc.tensor.matmul(ps, lhsT=ones_row, rhs=bo_sb, start=False, stop=True)
            x = ln_pool.tile([128, U], F32, tag="x")
            nc.scalar.activation(
                out=x, in_=ps, func=mybir.ActivationFunctionType.Relu
            )
            qn_t = ln_pool.tile([128, U], BF16, tag="qn")
            nc.sync.dma_start(out=qn_t, in_=p_qn[ti * 128:(ti + 1) * 128, :])
            nc.vector.tensor_tensor(out=x, in0=x, in1=qn_t, op=mybir.AluOpType.add)

            stats = ln_pool.tile([128, 6], F32, tag="st")
            nc.vector.bn_stats(out=stats, in_=x)
            mv = ln_pool.tile([128, 2], F32, tag="mv")
            nc.vector.bn_aggr(out=mv, in_=stats)
            rstd = ln_pool.tile([128, 1], F32, tag="rs")
            nc.scalar.activation(
                out=rstd,
                in_=mv[:, 1:2],
                func=mybir.ActivationFunctionType.Sqrt,
                bias=eps_sb,
            )
            nc.vector.reciprocal(out=rstd, in_=rstd)
            y = ln_pool.tile([128, U], F32, tag="y")
            nc.vector.tensor_scalar(
                out=y,
                in0=x,
                scalar1=mv[:, 0:1],
                scalar2=rstd,
                op0=mybir.AluOpType.subtract,
                op1=mybir.AluOpType.mult,
            )
            if apply_gamma_beta:
                nc.vector.tensor_tensor(
                    out=y, in0=y, in1=gam_bc, op=mybir.AluOpType.mult
                )
                nc.vector.tensor_tensor(
                    out=y, in0=y, in1=bet_bc, op=mybir.AluOpType.add
                )
            nc.sync.dma_start(out=p_out[ti * 128:(ti + 1) * 128, :], in_=y)

        cstack.close()

    split_excess_waits(nc)
    return nc


_NC_CACHE = {}


def _get_nc(apply_gamma_beta):
    key = bool(apply_gamma_beta)
    if key not in _NC_CACHE:
        _NC_CACHE[key] = build_nc(key)
    return _NC_CACHE[key]


def kernel(q, k, v, Wq, bq, Wk, bk, Wv, bv, Wo, bo, gamma, beta, pe_k, pe_v):
    q = np.asarray(q, np.float32)
    k = np.asarray(k, np.float32)
    v = np.asarray(v, np.float32)
    Wq = np.asarray(Wq, np.float32)
    Wk = np.asarray(Wk, np.float32)
    Wv = np.asarray(Wv, np.float32)
    Wo = np.asarray(Wo, np.float32)
    bq = np.asarray(bq, np.float32)
    bk = np.asarray(bk, np.float32)
    bv = np.asarray(bv, np.float32)
    bo = np.asarray(bo, np.float32)
    gamma = np.asarray(gamma, np.float32)
    beta = np.asarray(beta, np.float32)
    pe_k = np.asarray(pe_k, np.float32)
    pe_v = np.asarray(pe_v, np.float32)

    bf = ml_dtypes.bfloat16
    s = 1.0 / math.sqrt(DH)

    trivial = bool(np.all(gamma == 1.0) and np.all(beta == 0.0))
    nc = _get_nc(not trivial)

    shared = {
        "Wq8": (Wq * s).astype(bf),
        "Wk": Wk.astype(bf),
        "Wv": Wv.astype(bf),
        "Wo": Wo.astype(bf),
        "bq8": (bq * s).astype(np.float32).reshape(U, 1),
        "bk": bk.reshape(U, 1),
        "bv_row": bv.astype(bf).reshape(1, U),
        "bo_row": (bo + np.tile(pe_v[0], H) @ Wo).astype(bf).reshape(1, U),
        "pe_k0t": np.tile(pe_k[0], H).astype(np.float32).reshape(U, 1),
        "dpe_kT": np.ascontiguousarray((pe_k[1:17] - pe_k[0]).T).astype(bf),
        "dpe_v": (pe_v[1:17] - pe_v[0]).astype(bf),
        "gamma_r": gamma.reshape(1, U).astype(np.float32),
        "beta_r": beta.reshape(1, U).astype(np.float32),
    }

    in_maps = []
    for b_i in range(B):
        m = dict(shared)
        m["qT"] = np.ascontiguousarray(q[b_i].T).astype(bf)
        m["kT"] = np.ascontiguousarray(k[b_i].T).astype(bf)
        m["vT"] = np.ascontiguousarray(v[b_i].T).astype(bf)
        m["qn"] = q[b_i].astype(bf)
        in_maps.append(m)

    res = run_bass_kernel_spmd(nc, in_maps, core_ids=list(range(B)))
    global LAST_RESULT
    LAST_RESULT = res
    out = np.stack([res.results[b_i]["out"] for b_i in range(B)], axis=0)
    return out


LAST_RESULT = None


# revision 20
# speedup vs baseline: 15.0877x; 15.0877x over previous
"""Trainium2 Bass kernel for nn_MultiHeadAttention_87110526697836.

Strategy: data-parallel over batch B=8 across the 8 NeuronCores (one batch
element per core, no collectives). Per core, a causal MHA layer with
relative-position biases (max_dist=16), fused softmax, output projection,
residual and LayerNorm.

Layout: activations kept transposed ([U, T], "T-plan") so that
  - QKV projections use natural weight layouts,
  - scores are computed as [s-partition, t-free] tiles (kh stationary),
  - P@V consumes the score tiles directly (no P transposes),
  - per-t softmax scalars (1/den) are applied after P@V where t is back on
    partitions.
The relative-position band (17 diagonals) is handled via DRAM "skew
staging" round trips batched per head: diagonal bands of the per-head
[128, 8, 1040] probability tile are extracted / corrected / scattered with
strided DMA access patterns on flat DRAM buffers.
"""

import math

import numpy as np
import ml_dtypes

import bass_rust
import concourse.bass as bass
import concourse.mybir as mybir
import concourse.tile as tile
from concourse.bass_utils import run_bass_kernel_spmd

F32 = mybir.dt.float32
BF16 = mybir.dt.bfloat16

H = 8
M = 16
U = 512
DH = U // H        # 64
T = 1024
B = 8
EPS = 1e-3
NT = T // 128      # 8 t-chunks / s-chunks
WTW = 1040         # per-chunk padded width of the w tile (128*8 + 16)
CSTRIDE = WTW + 128  # step between (c, s0_c) anchors inside a wT_all tile
EXT_ROWS, EXT_W = 160, 144
RW = NT * EXT_W          # interleaved staging row width (1152)
HSTG = EXT_ROWS * RW     # per-head staging size (184320)
CORNER_OFF = 1152 * 128 - 144 + 128   # read2/scat2 offset within a head block
BSTG = 4096              # per-head corner staging size

# ---------------------------------------------------------------------------
# Tile-framework workarounds: walrus accepts a limited number of sync-wait
# commands per instruction; split excess waits onto same-engine NOPs.
# ---------------------------------------------------------------------------


def _patched_drain_and_barrier(self, tick_clock, wait_clock):
    from bass_rust import ScopedClock

    nc = self.nc
    nop_inst = nc.sync.nop()
    wait_clock.add_sem_waits(nop_inst.ins, ScopedClock({None: tick_clock.global_clock}))
    si = nop_inst.ins.sync_info
    if si is not None and si.on_wait and len(si.on_wait) > 1:
        waits = list(si.on_wait)
        nop_inst.ins.sync_info = bass_rust.SyncInfo(
            on_wait=[waits[0]], on_update=list(si.on_update or [])
        )
        for w in waits[1:]:
            extra = nc.sync.nop()
            extra.ins.sync_info = bass_rust.SyncInfo(on_wait=[w], on_update=[])
    nc.sync.drain()
    nc.all_engine_barrier()
    popped = nc._tile_sem_poison_stack.pop()
    assert popped is self._sem_poison
    nc.clear_and_free_semaphores(list(self.sems.allocated().values()))
    nc.all_engine_barrier()


tile.TileContext._drain_and_barrier = _patched_drain_and_barrier


def split_excess_waits(nc, limit=1):
    counter = 0
    for f in nc.m.functions:
        for bb in f.blocks:
            insts = bb.instructions
            out = []
            changed = False
            for ins in insts:
                si = ins.sync_info
                if si is not None and si.on_wait is not None and len(si.on_wait) > limit:
                    waits = list(si.on_wait)
                    extra, keep = waits[:-limit], waits[-limit:]
                    for w in extra:
                        counter += 1
                        nop = mybir.InstNoOp(name=f"waitsplit-{counter}", ins=[], outs=[])
                        nop.engine = ins.engine
                        nop.sync_info = bass_rust.SyncInfo(on_wait=[w], on_update=[])
                        nc.inst_map[nop.name] = nop
                        out.append(nop)
                    ins.sync_info = bass_rust.SyncInfo(
                        on_wait=keep, on_update=list(si.on_update or [])
                    )
                    changed = True
                out.append(ins)
            if changed:
                bb.instructions = out
    return counter


def dram_ap(t, offset, dims):
    return bass.AP(tensor=t, offset=offset, ap=[list(d) for d in dims])


# ---------------------------------------------------------------------------
# Kernel builder
# ---------------------------------------------------------------------------


def build_nc(apply_gamma_beta: bool):
    nc = bass.Bass(num_swdge_queues=4)

    p_qT = nc.declare_dram_parameter("qT", [U, T], BF16, isOutput=False)
    p_kT = nc.declare_dram_parameter("kT", [U, T], BF16, isOutput=False)
    p_vT = nc.declare_dram_parameter("vT", [U, T], BF16, isOutput=False)
    p_qn = nc.declare_dram_parameter("qn", [T, U], BF16, isOutput=False)
    p_Wq = nc.declare_dram_parameter("Wq8", [U, U], BF16, isOutput=False)
    p_Wk = nc.declare_dram_parameter("Wk", [U, U], BF16, isOutput=False)
    p_Wv = nc.declare_dram_parameter("Wv", [U, U], BF16, isOutput=False)
    p_Wo = nc.declare_dram_parameter("Wo", [U, U], BF16, isOutput=False)
    p_bq = nc.declare_dram_parameter("bq8", [U, 1], F32, isOutput=False)
    p_bk = nc.declare_dram_parameter("bk", [U, 1], F32, isOutput=False)
    p_bv = nc.declare_dram_parameter("bv_row", [1, U], BF16, isOutput=False)
    p_bo = nc.declare_dram_parameter("bo_row", [1, U], BF16, isOutput=False)
    p_pk0 = nc.declare_dram_parameter("pe_k0t", [U, 1], F32, isOutput=False)
    p_dpk = nc.declare_dram_parameter("dpe_kT", [DH, M], BF16, isOutput=False)
    p_dpv = nc.declare_dram_parameter("dpe_v", [M, DH], BF16, isOutput=False)
    p_m8 = nc.declare_dram_parameter("mask8", [128, 1], F32, isOutput=False)
    p_gam = nc.declare_dram_parameter("gamma_r", [1, U], F32, isOutput=False)
    p_bet = nc.declare_dram_parameter("beta_r", [1, U], F32, isOutput=False)
    p_out = nc.declare_dram_parameter("out", [T, U], F32, isOutput=True)

    ext_stage = nc.dram_tensor("ext_stage", [H * HSTG], BF16)
    corr_stage = nc.dram_tensor("corr_stage", [H * HSTG], BF16)
    cornerB = nc.dram_tensor("cornerB", [H * BSTG], BF16)
    g_stage = nc.dram_tensor("g_stage", [H * 16384], BF16)
    wb_stage = nc.dram_tensor("wb_stage", [H * 16384], BF16)
    rden_stage = nc.dram_tensor("rden_stage", [H * T], F32)

    with tile.TileContext(nc) as tc:
        import contextlib

        cstack = contextlib.ExitStack()
        consts = cstack.enter_context(tc.tile_pool(name="consts", bufs=1))
        emb_pool = cstack.enter_context(tc.tile_pool(name="emb", bufs=1))
        wt_pool = cstack.enter_context(tc.tile_pool(name="wt", bufs=2))
        band_pool = cstack.enter_context(tc.tile_pool(name="band", bufs=2))
        g_pool = cstack.enter_context(tc.tile_pool(name="g", bufs=3))
        mrg_pool = cstack.enter_context(tc.tile_pool(name="mrg", bufs=1))
        nrm_pool = cstack.enter_context(tc.tile_pool(name="nrm", bufs=2))
        ln_pool = cstack.enter_context(tc.tile_pool(name="ln", bufs=3))
        y_pool = cstack.enter_context(tc.tile_pool(name="ypool", bufs=1))

        ps_a = cstack.enter_context(tc.tile_pool(name="ps_a", bufs=2, space="PSUM"))
        ps_sc = cstack.enter_context(tc.tile_pool(name="ps_sc", bufs=2, space="PSUM"))
        ps_o = cstack.enter_context(tc.tile_pool(name="ps_o", bufs=2, space="PSUM"))

        # ---- staging zero-init (batched) ---------------------------------
        ZW = 2304
        ztile = consts.tile([128, ZW], BF16, tag="zz")
        nc.vector.memset(ztile, 0.0)
        total = H * HSTG
        chunk = 128 * ZW
        assert total % chunk == 0
        for i, off in enumerate(range(0, total, chunk)):
            nc.gpsimd.dma_start(
                out=dram_ap(ext_stage, off, [[ZW, 128], [1, ZW]]),
                in_=ztile,
            )
            nc.gpsimd.dma_start(
                out=dram_ap(corr_stage, off, [[ZW, 128], [1, ZW]]),
                in_=ztile,
            )
        nc.gpsimd.dma_start(
            out=dram_ap(cornerB, 0, [[256, 128], [1, 256]]), in_=ztile[:, 0:256]
        )

        # ---- constant loads (batched) ------------------------------------
        def load_w(param, name):
            t_ = consts.tile([128, 4, U], BF16, tag=f"w{name}")
            nc.sync.dma_start(
                out=t_, in_=dram_ap(param, 0, [[U, 128], [128 * U, 4], [1, U]])
            )
            return t_

        Wq_sb = load_w(p_Wq, "q")
        Wk_sb = load_w(p_Wk, "k")
        Wv_sb = load_w(p_Wv, "v")
        Wo_sb = load_w(p_Wo, "o")

        def load_xT(param, name):
            t_ = consts.tile([128, 4, T], BF16, tag=f"x{name}")
            nc.sync.dma_start(
                out=t_, in_=dram_ap(param, 0, [[T, 128], [128 * T, 4], [1, T]])
            )
            return t_

        qT_sb = load_xT(p_qT, "q")
        kT_sb = load_xT(p_kT, "k")
        vT_sb = load_xT(p_vT, "v")

        bq_sb = consts.tile([128, 4], F32, tag="bq")
        nc.sync.dma_start(out=bq_sb, in_=dram_ap(p_bq, 0, [[1, 128], [128, 4]]))
        bk_sb = consts.tile([128, 4], F32, tag="bk")
        nc.sync.dma_start(out=bk_sb, in_=dram_ap(p_bk, 0, [[1, 128], [128, 4]]))
        pk0_sb = consts.tile([128, 4], F32, tag="pk0")
        nc.sync.dma_start(out=pk0_sb, in_=dram_ap(p_pk0, 0, [[1, 128], [128, 4]]))

        bv_sb = consts.tile([1, U], BF16, tag="bv")
        nc.sync.dma_start(out=bv_sb, in_=p_bv[:, :])
        bo_sb = consts.tile([1, U], BF16, tag="bo")
        nc.sync.dma_start(out=bo_sb, in_=p_bo[:, :])
        dpk_sb = consts.tile([128, M], BF16, tag="dpk")
        nc.sync.dma_start(out=dpk_sb[0:DH, :], in_=p_dpk[:, :])
        nc.sync.dma_start(out=dpk_sb[DH:128, :], in_=p_dpk[:, :])
        dpv_sb = consts.tile([M, DH], BF16, tag="dpv")
        nc.sync.dma_start(out=dpv_sb, in_=p_dpv[:, :])
        m8_sb = consts.tile([128, 1], F32, tag="m8")
        nc.sync.dma_start(out=m8_sb, in_=p_m8[:, :])

        if apply_gamma_beta:
            gam_bc = consts.tile([128, U], F32, tag="gambc")
            nc.sync.dma_start(out=gam_bc, in_=dram_ap(p_gam, 0, [[0, 128], [1, U]]))
            bet_bc = consts.tile([128, U], F32, tag="betbc")
            nc.sync.dma_start(out=bet_bc, in_=dram_ap(p_bet, 0, [[0, 128], [1, U]]))

        ones_row = consts.tile([1, 128], BF16, tag="ones")
        nc.vector.memset(ones_row, 1.0)
        eps_sb = consts.tile([128, 1], F32, tag="eps")
        nc.vector.memset(eps_sb, EPS)

        qn_sb = consts.tile([128, NT, U], BF16, tag="qn")
        nc.sync.dma_start(
            out=qn_sb, in_=dram_ap(p_qn, 0, [[U, 128], [128 * U, NT], [1, U]])
        )

        # ---- phase 1: projections ----------------------------------------
        q_embT = emb_pool.tile([128, 4, T], BF16, tag="qe")
        k_embT = emb_pool.tile([128, 4, T], BF16, tag="ke")
        v_pad = emb_pool.tile([128, NT, H, 66], BF16, tag="vp")

        for co in range(4):
            for th in range(2):
                ps = ps_a.tile([128, 512], F32, tag="proj")
                for ci in range(4):
                    nc.tensor.matmul(
                        ps,
                        lhsT=Wq_sb[:, ci, co * 128:(co + 1) * 128],
                        rhs=qT_sb[:, ci, th * 512:(th + 1) * 512],
                        start=(ci == 0),
                        stop=(ci == 3),
                    )
                nc.scalar.activation(
                    out=q_embT[:, co, th * 512:(th + 1) * 512],
                    in_=ps,
                    func=mybir.ActivationFunctionType.Relu,
                    bias=bq_sb[:, co:co + 1],
                )
        for co in range(4):
            for th in range(2):
                ps = ps_a.tile([128, 512], F32, tag="proj")
                for ci in range(4):
                    nc.tensor.matmul(
                        ps,
                        lhsT=Wk_sb[:, ci, co * 128:(co + 1) * 128],
                        rhs=kT_sb[:, ci, th * 512:(th + 1) * 512],
                        start=(ci == 0),
                        stop=(ci == 3),
                    )
                nc.scalar.activation(
                    out=k_embT[:, co, th * 512:(th + 1) * 512],
                    in_=ps,
                    func=mybir.ActivationFunctionType.Relu,
                    bias=bk_sb[:, co:co + 1],
                )
            # fold pe_k[0] into keys (handles the r0 part of the rel-k bias)
            nc.vector.tensor_scalar_add(
                out=k_embT[:, co, :], in0=k_embT[:, co, :],
                scalar1=pk0_sb[:, co:co + 1],
            )

        for ti in range(NT):
            ps = ps_a.tile([128, 512], F32, tag="proj")
            for ci in range(4):
                nc.tensor.matmul(
                    ps,
                    lhsT=vT_sb[:, ci, ti * 128:(ti + 1) * 128],
                    rhs=Wv_sb[:, ci, :],
                    start=(ci == 0),
                    stop=False,
                )
            nc.tensor.matmul(ps, lhsT=ones_row, rhs=bv_sb, start=False, stop=True)
            nc.scalar.activation(
                out=v_pad[:, ti, :, 0:64],
                in_=ps.rearrange("p (a b) -> p a b", a=H),
                func=mybir.ActivationFunctionType.Relu,
            )
        nc.vector.memset(v_pad[:, :, :, 64:65], 1.0)

        mergedT = mrg_pool.tile([128, 4, T], BF16, tag="mg")

        # ---- phase 2: attention per head ---------------------------------
        for h in range(H):
            hb = 64 * (h % 2)
            qhT = q_embT[hb:hb + 64, h // 2, :]
            khT = k_embT[hb:hb + 64, h // 2, :]

            # G = exp(qh . (pe_k[c]-pe_k[0])), c=1..16  -> [16, T] bf16
            g_sb = g_pool.tile([M, T], BF16, tag="gsb")
            gps = ps_sc.tile([128, T], F32, tag="scores")
            for th in range(2):
                nc.tensor.matmul(
                    gps[0:M, th * 512:(th + 1) * 512],
                    lhsT=dpk_sb[hb:hb + 64, :],
                    rhs=qhT[:, th * 512:(th + 1) * 512],
                    start=True,
                    stop=True,
                    skip_group_check=True,
                )
            nc.scalar.activation(
                out=g_sb, in_=gps[0:M, :], func=mybir.ActivationFunctionType.Exp
            )

            # scores + exp: one [128, 8, WTW] w tile per head
            wt_t = wt_pool.tile([128, NT, WTW], BF16, tag="wt")
            for c in range(NT):
                s0 = c * 128
                ps = ps_sc.tile([128, T], F32, tag="scores")
                for th in range(2):
                    lo = max(th * 512, s0)
                    hi = (th + 1) * 512
                    if lo >= hi:
                        continue
                    nc.tensor.matmul(
                        ps[:, lo:hi],
                        lhsT=khT[:, s0:s0 + 128],
                        rhs=qhT[:, lo:hi],
                        start=True,
                        stop=True,
                        skip_group_check=True,
                    )
                nc.scalar.activation(
                    out=wt_t[:, c, s0:T],
                    in_=ps[:, s0:T],
                    func=mybir.ActivationFunctionType.Exp,
                )
            # pad columns of the last chunk are never computed; zero them
            # so the staging write doesn't move uninitialized bytes
            nc.vector.memset(wt_t[:, NT - 1, T:WTW], 0.0)
            # causal mask for all 8 diagonal blocks in one op:
            # zero wt_t[p, c, s0_c + x] where x < p  (iota = x - p)
            diag_view = bass.AP(
                tensor=wt_t.tensor,
                offset=wt_t.offset,
                ap=[list(wt_t.ap[0]), [CSTRIDE, NT], [1, 128]],
            )
            nc.gpsimd.affine_select(
                out=diag_view,
                in_=diag_view,
                compare_op=mybir.AluOpType.is_ge,
                fill=0.0,
                base=0,
                channel_multiplier=-1,
                pattern=[[0, NT], [1, 128]],
            )
            # stage band windows of all 8 chunks: one DMA
            # (interleaved layout: flat = row*RW + c*EXT_W + x, row = s'+15)
            hbase = h * HSTG
            win_view = bass.AP(
                tensor=wt_t.tensor,
                offset=wt_t.offset,
                ap=[list(wt_t.ap[0]), [CSTRIDE, NT], [1, EXT_W]],
            )
            nc.gpsimd.dma_start(
                out=dram_ap(ext_stage, hbase + 15 * RW, [[RW, 128], [1, RW]]),
                in_=win_view,
            )

            # band extraction, p = 8*jj + ti interleaved partitions
            bE = band_pool.tile([128, 128], BF16, tag="bE")
            for hh in range(2):
                nc.gpsimd.dma_start(
                    out=bE[:, hh * 64:(hh + 1) * 64],
                    in_=dram_ap(
                        ext_stage, hbase + hh * 64 * (RW + 1),
                        [[EXT_W, 128], [RW + 1, 64]],
                    ),
                )
            bE2 = band_pool.tile([128, 16], BF16, tag="bE2")
            nc.gpsimd.dma_start(
                out=bE2,
                in_=dram_ap(
                    ext_stage, hbase + CORNER_OFF, [[EXT_W, 128], [RW + 1, 16]]
                ),
            )
            # zero the ti==0 partitions (no previous chunk) and fold corners
            nc.vector.tensor_scalar_mul(out=bE2, in0=bE2, scalar1=m8_sb)
            nc.vector.tensor_tensor(
                out=bE[:, 0:16], in0=bE[:, 0:16], in1=bE2, op=mybir.AluOpType.add
            )
            # G reshaped to p-layout via DRAM (g_sb[jj, ti*128+t'] -> [8jj+ti, t'])
            gb = h * 16384
            nc.gpsimd.dma_start(
                out=dram_ap(g_stage, gb, [[1024, M], [1, 1024]]),
                in_=g_sb.rearrange("p (a b) -> p a b", a=NT),
            )
            g_p = band_pool.tile([128, 128], BF16, tag="gp")
            nc.gpsimd.dma_start(
                out=g_p, in_=dram_ap(g_stage, gb, [[128, 128], [1, 128]])
            )
            wband_p = band_pool.tile([128, 128], BF16, tag="wbp")
            nc.vector.tensor_tensor(
                out=wband_p, in0=bE, in1=g_p, op=mybir.AluOpType.mult
            )
            corr = band_pool.tile([128, 128], BF16, tag="corr")
            nc.vector.tensor_tensor(
                out=corr, in0=wband_p, in1=bE, op=mybir.AluOpType.subtract
            )
            corr2 = band_pool.tile([128, 16], BF16, tag="corr2")
            nc.vector.tensor_scalar_mul(out=corr2, in0=corr[:, 0:16], scalar1=m8_sb)
            # wband back to [16, 1024] layout for the rel-v matmul
            wb_b = h * 16384
            nc.gpsimd.dma_start(
                out=dram_ap(wb_stage, wb_b, [[128, 128], [1, 128]]), in_=wband_p
            )
            wband = band_pool.tile([M, T], BF16, tag="wb")
            nc.gpsimd.dma_start(
                out=wband, in_=dram_ap(wb_stage, wb_b, [[1024, M], [1, 1024]])
            )
            # scatter corrections
            for hh in range(2):
                nc.gpsimd.dma_start(
                    out=dram_ap(
                        corr_stage, hbase + hh * 64 * (RW + 1),
                        [[EXT_W, 128], [RW + 1, 64]],
                    ),
                    in_=corr[:, hh * 64:(hh + 1) * 64],
                )
            bb = h * BSTG + 128   # front pad (>=16) for the p==0 wrap
            nc.gpsimd.dma_start(
                out=dram_ap(cornerB, bb - 16, [[16, 128], [129, 16]]),
                in_=corr2,
            )
            # dense correction read-back + add
            cd = band_pool.tile([128, NT, EXT_W + 2], BF16, tag="cd")
            nc.gpsimd.dma_start(
                out=cd[:, :, 0:EXT_W],
                in_=dram_ap(corr_stage, hbase + 15 * RW, [[RW, 128], [1, RW]]),
            )
            nc.gpsimd.dma_start(
                out=cd[113:128, :, 128:144],
                in_=dram_ap(cornerB, bb, [[128, 15], [16, 8], [1, 16]]),
            )
            win_view2 = bass.AP(
                tensor=wt_t.tensor,
                offset=wt_t.offset,
                ap=[list(wt_t.ap[0]), [CSTRIDE, NT], [1, EXT_W]],
            )
            nc.vector.tensor_tensor(
                out=win_view2, in0=win_view2, in1=cd[:, :, 0:EXT_W],
                op=mybir.AluOpType.add,
            )

            # P @ [V | 1] + rel-v band; normalize into mergedT
            nrm = nrm_pool.tile([1, T], F32, tag="rden")
            po_list = []
            for th in range(2):
                po = ps_o.tile([65, 512], F32, tag="po")
                po_list.append(po)
                nmax = 4 * (th + 1)
                for c in range(nmax):
                    s0 = c * 128
                    lo = max(0, s0 - th * 512)
                    nc.tensor.matmul(
                        po[:, lo:512],
                        lhsT=v_pad[:, c, h, 0:65],
                        rhs=wt_t[:, c, th * 512 + lo:(th + 1) * 512],
                        start=(c == 0),
                        stop=False,
                        skip_group_check=True,
                    )
                for tl in range(4):
                    ti = th * 4 + tl
                    nc.tensor.matmul(
                        po[0:64, tl * 128:(tl + 1) * 128],
                        lhsT=dpv_sb,
                        rhs=wband[:, ti * 128:(ti + 1) * 128],
                        start=False,
                        stop=(tl == 3),
                        skip_group_check=True,
                    )
                nc.vector.reciprocal(
                    out=nrm[:, th * 512:(th + 1) * 512], in_=po[64:65, :]
                )
            nc.gpsimd.dma_start(
                out=dram_ap(rden_stage, h * T, [[T, 1], [1, T]]), in_=nrm,
            )
            nrmb = nrm_pool.tile([64, T], F32, tag="rdenb")
            nc.gpsimd.dma_start(
                out=nrmb, in_=dram_ap(rden_stage, h * T, [[0, 64], [1, T]]),
            )
            for th in range(2):
                nc.vector.tensor_tensor(
                    out=mergedT[hb:hb + 64, h // 2, th * 512:(th + 1) * 512],
                    in0=po_list[th][0:64, :],
                    in1=nrmb[:, th * 512:(th + 1) * 512],
                    op=mybir.AluOpType.mult,
                )

        # ---- phase 3: output projection + residual + layernorm ----------
        y_all = y_pool.tile([128, NT, U], F32, tag="yall")
        for ti in range(NT):
            ps = ps_a.tile([128, 512], F32, tag="proj")
            for ci in range(4):
                nc.tensor.matmul(
                    ps,
                    lhsT=mergedT[:, ci, ti * 128:(ti + 1) * 128],
                    rhs=Wo_sb[:, ci, :],
                    start=(ci == 0),
                    stop=False,
                )
            nc.tensor.matmul(ps, lhsT=ones_row, rhs=bo_sb, start=False, stop=True)
            x = ln_pool.tile([128, U], F32, tag="x")
            nc.scalar.activation(
                out=x, in_=ps, func=mybir.ActivationFunctionType.Relu
            )
            nc.vector.tensor_tensor(
                out=x, in0=x, in1=qn_sb[:, ti, :], op=mybir.AluOpType.add
            )
            stats = ln_pool.tile([128, 6], F32, tag="st")
            nc.vector.bn_stats(out=stats, in_=x)
            mv = ln_pool.tile([128, 2], F32, tag="mv")
            nc.vector.bn_aggr(out=mv, in_=stats)
            rstd = ln_pool.tile([128, 1], F32, tag="rs")
            nc.scalar.activation(
                out=rstd,
                in_=mv[:, 1:2],
                func=mybir.ActivationFunctionType.Sqrt,
                bias=eps_sb,
            )
            nc.vector.reciprocal(out=rstd, in_=rstd)
            y = y_all[:, ti, :]
            nc.vector.tensor_scalar(
                out=y,
                in0=x,
                scalar1=mv[:, 0:1],
                scalar2=rstd,
                op0=mybir.AluOpType.subtract,
                op1=mybir.AluOpType.mult,
            )
            if apply_gamma_beta:
                nc.vector.tensor_tensor(
                    out=y, in0=y, in1=gam_bc, op=mybir.AluOpType.mult
                )
                nc.vector.tensor_tensor(
                    out=y, in0=y, in1=bet_bc, op=mybir.AluOpType.add
                )
        nc.sync.dma_start(
            out=dram_ap(p_out, 0, [[U, 128], [128 * U, NT], [1, U]]), in_=y_all
        )

        cstack.close()

    split_excess_waits(nc)
    return nc


_NC_CACHE = {}


def _get_nc(apply_gamma_beta):
    key = bool(apply_gamma_beta)
    if key not in _NC_CACHE:
        _NC_CACHE[key] = build_nc(key)
    return _NC_CACHE[key]


def kernel(q, k, v, Wq, bq, Wk, bk, Wv, bv, Wo, bo, gamma, beta, pe_k, pe_v):
    q = np.asarray(q, np.float32)
    k = np.asarray(k, np.float32)
    v = np.asarray(v, np.float32)
    Wq = np.asarray(Wq, np.float32)
    Wk = np.asarray(Wk, np.float32)
    Wv = np.asarray(Wv, np.float32)
    Wo = np.asarray(Wo, np.float32)
    bq = np.asarray(bq, np.float32)
    bk = np.asarray(bk, np.float32)
    bv = np.asarray(bv, np.float32)
    bo = np.asarray(bo, np.float32)
    gamma = np.asarray(gamma, np.float32)
    beta = np.asarray(beta, np.float32)
    pe_k = np.asarray(pe_k, np.float32)
    pe_v = np.asarray(pe_v, np.float32)

    bf = ml_dtypes.bfloat16
    s = 1.0 / math.sqrt(DH)

    trivial = bool(np.all(gamma == 1.0) and np.all(beta == 0.0))
    nc = _get_nc(not trivial)

    shared = {
        "Wq8": (Wq * s).astype(bf),
        "Wk": Wk.astype(bf),
        "Wv": Wv.astype(bf),
        "Wo": Wo.astype(bf),
        "bq8": (bq * s).astype(np.float32).reshape(U, 1),
        "bk": bk.reshape(U, 1),
        "bv_row": bv.astype(bf).reshape(1, U),
        "bo_row": (bo + np.tile(pe_v[0], H) @ Wo).astype(bf).reshape(1, U),
        "pe_k0t": np.tile(pe_k[0], H).astype(np.float32).reshape(U, 1),
        "mask8": (np.arange(128) % 8 != 0).astype(np.float32).reshape(128, 1),
        "dpe_kT": np.ascontiguousarray((pe_k[1:17] - pe_k[0]).T).astype(bf),
        "dpe_v": (pe_v[1:17] - pe_v[0]).astype(bf),
        "gamma_r": gamma.reshape(1, U).astype(np.float32),
        "beta_r": beta.reshape(1, U).astype(np.float32),
    }

    in_maps = []
    for b_i in range(B):
        m = dict(shared)
        m["qT"] = np.ascontiguousarray(q[b_i].T).astype(bf)
        m["kT"] = np.ascontiguousarray(k[b_i].T).astype(bf)
        m["vT"] = np.ascontiguousarray(v[b_i].T).astype(bf)
        m["qn"] = q[b_i].astype(bf)
        in_maps.append(m)

    res = run_bass_kernel_spmd(nc, in_maps, core_ids=list(range(B)))
    global LAST_RESULT
    LAST_RESULT = res
    out = np.stack([res.results[b_i]["out"] for b_i in range(B)], axis=0)
    return out


LAST_RESULT = None
